# revision 21
# baseline (speedup 1.0000x reference)
"""3-layer GAT on trn2, 8 NeuronCores, edge-parallel with dst-range sharding.

v2 (bf16 edge phase, window-batched ops):
- Dense phase: W is host-augmented with block-diag-projected attention vectors
  (W_aug = [W | W@As_bd | W@Ad_bd]), so per-node h, alpha_src, alpha_dst all
  fall out of ONE matmul. Rows [h | alpha_src] are packed bf16 into a DRAM
  table and AllGathered; alpha_dst^T is kept local in SBUF ([16, SHARD] bf16)
  for gpsimd ap_gather.
- Edge phase per 125-dst window: one dma_gather pulls bf16 rows for the
  window's edges; one Pool ap_gather pulls per-edge alpha_dst (transposed
  [16, kw]); per 128-edge chunk a PE transpose flips it to per-edge layout in
  PSUM. Scores st=as+ad, Lrelu (Act, alpha=0.2), Exp (Act) -> p written
  straight into the rhs tile's tail columns; one window-wide DVE multiply
  weights the h columns by p; one window-wide DVE is_equal builds the one-hot
  scatter matrix from a host dstloc stream; per-chunk bf16 matmuls
  scatter-accumulate [sum p*h | sum p] into PSUM. Epilogue divides (+bias,
  relu) and transposes the activation for the next dense phase.
"""
import os, sys
for _p in ('/opt/trn_rl_repo', '/root/.axon_site/_ro/trn_rl_repo'):
    if os.path.isdir(_p) and _p not in sys.path:
        sys.path.insert(0, _p)

import numpy as np

import concourse.bacc as bacc
import concourse.tile as tile
from concourse import bass, mybir
from concourse import bass_utils

N = 20000
E = 320000
HID = 64
HEADS = 4
OUT_CH = 64
NEG = 0.2
C = 8
SHARD = N // C          # 2500
WIN = 125               # dst nodes per window
NW = SHARD // WIN       # 20
P = 128

# fin, fout, heads, table row width (bf16 elems; row*2 % 256 == 0)
LAYERS = [
    dict(fin=64,  fout=256, heads=4, row=384),
    dict(fin=256, fout=256, heads=4, row=384),
    dict(fin=256, fout=64,  heads=1, row=128),
]

AX = mybir.AxisListType
ALU = mybir.AluOpType
ACTF = mybir.ActivationFunctionType
F32 = mybir.dt.float32
BF16 = mybir.dt.bfloat16
F8 = mybir.dt.float8e4
I16 = mybir.dt.int16


def _host_prep(edge_index):
    """Per-core gather-idx / apg-idx / dstloc streams and shared window sizes."""
    src = np.asarray(edge_index[0], dtype=np.int64)
    dst = np.asarray(edge_index[1], dtype=np.int64)
    per_core = []   # (srcs, dstloc) per (core, window)
    counts = np.zeros((C, NW), dtype=np.int64)
    for c in range(C):
        m = (dst >= c * SHARD) & (dst < (c + 1) * SHARD)
        es, ed = src[m], dst[m] - c * SHARD
        order = np.argsort(ed, kind='stable')
        es, ed = es[order], ed[order]
        w = ed // WIN
        wins = []
        for wi in range(NW):
            sel = w == wi
            wins.append((es[sel], ed[sel] - wi * WIN))
            counts[c, wi] = sel.sum()
        per_core.append(wins)
    kws = (np.ceil(counts.max(axis=0) / P).astype(np.int64) * P)
    kws = np.maximum(kws, P)
    tot = int(kws.sum())
    idx_all, oh_all, ohT_all = [], [], []
    for c in range(C):
        idx_mat = np.zeros((16, tot // 16), dtype=np.int16)
        dl_mat = np.full((P, tot // P), WIN, dtype=np.int64)
        icol = ccol = 0
        for wi in range(NW):
            kw = int(kws[wi])
            es, dl = per_core[c][wi]
            n = len(es)
            sp = np.zeros(kw, dtype=np.int16)
            dp = np.full(kw, WIN, dtype=np.int64)
            sp[:n] = es.astype(np.int16)
            dp[:n] = dl
            idx_mat[:, icol:icol + kw // 16] = sp.reshape(-1, 16).T
            dl_mat[:, ccol:ccol + kw // P] = dp.reshape(-1, P).T
            icol += kw // 16
            ccol += kw // P
        idx_all.append(np.tile(idx_mat, (8, 1)))
        # host-precomputed one-hot scatter matrix [128, nchunks*WIN] and its
        # per-chunk transpose [128(dst), nchunks*128]
        oh = np.zeros((P, (tot // P) * WIN), dtype=np.float32)
        ohT = np.zeros((P, tot), dtype=np.float32)
        pi, ci = np.nonzero(dl_mat < WIN)
        oh[pi, ci * WIN + dl_mat[pi, ci]] = 1.0
        ohT[dl_mat[pi, ci], ci * P + pi] = 1.0
        oh_all.append(oh)
        ohT_all.append(ohT)
    return tuple(int(k) for k in kws), idx_all, oh_all, ohT_all


def build(kws, timing_reps=0):
    """Builds the SPMD bass module. kws: per-window padded edge counts."""
    tot = sum(kws)
    tw_max = max(kws) // P
    nc = bacc.Bacc("TRN2", target_bir_lowering=False, debug=False, num_devices=C)

    # ---- DRAM I/O ----
    d_xT = nc.dram_tensor("xT_own", [HID, SHARD], BF16, kind="ExternalInput")
    # augmented weights: [fin, fout + 2*heads] = [W | W@As_bd | W@Ad_bd]
    d_W = [nc.dram_tensor(f"W{l+1}a", [LAYERS[l]['fin'],
                                       LAYERS[l]['fout'] + 2 * LAYERS[l]['heads']],
                          BF16, kind="ExternalInput") for l in range(3)]
    d_b = [nc.dram_tensor(f"br{l+1}", [P, LAYERS[l]['fout']], F32,
                          kind="ExternalInput") for l in range(3)]
    d_identb = nc.dram_tensor("identb", [P, P], BF16, kind="ExternalInput")
    d_idx = nc.dram_tensor("gat_idx", [P, tot // 16], I16, kind="ExternalInput")
    d_oh = nc.dram_tensor("oh_all", [P, (tot // P) * WIN], F8,
                          kind="ExternalInput")
    d_ohT = nc.dram_tensor("ohT_all", [P, tot], F8, kind="ExternalInput")
    d_out = nc.dram_tensor("out", [SHARD, OUT_CH], F32, kind="ExternalOutput")
    if timing_reps:
        d_tok = nc.dram_tensor("tok", [1, 32], F32, kind="ExternalInput")
        d_toko = nc.dram_tensor("tok_out", [1, 32], F32, kind="ExternalOutput")

    tabs = []
    for l, cfg in enumerate(LAYERS):
        s = nc.dram_tensor(f"tab{l+1}s", [SHARD, cfg['row']], BF16)
        f = nc.dram_tensor(f"tab{l+1}f", [N, cfg['row']], BF16, addr_space="Shared")
        tabs.append((s, f))

    with tile.TileContext(nc) as tc:
        with tc.tile_pool(name="const", bufs=1) as cp, \
             tc.tile_pool(name="rowp", bufs=3) as rowp, \
             tc.tile_pool(name="gp", bufs=3) as gp, \
             tc.tile_pool(name="sp", bufs=3) as sp, \
             tc.tile_pool(name="rp", bufs=3) as rp, \
             tc.tile_pool(name="op", bufs=3) as op_, \
             tc.tile_pool(name="ps", bufs=1, space="PSUM") as pp:

            # ---- persistent SBUF ----
            identb = cp.tile([P, P], BF16)
            nc.sync.dma_start(identb[:], d_identb[:, :])
            idx_sb = cp.tile([P, tot // 16], I16)
            oh_sb = cp.tile([P, (tot // P) * WIN], F8)
            ohT_sb = cp.tile([P, tot], F8)
            nc.sync.dma_start(idx_sb[:], d_idx[:, :])
            nc.sync.dma_start(oh_sb[:], d_oh[:, :])
            nc.sync.dma_start(ohT_sb[:], d_ohT[:, :])
            xT = cp.tile([HID, SHARD], BF16)
            nc.sync.dma_start(xT[:], d_xT[:, :])
            Wt, bt = [], []
            for l, cfg in enumerate(LAYERS):
                fin = cfg['fin']
                fa = cfg['fout'] + 2 * cfg['heads']
                chunks = []
                for kc in range(0, fin, P):
                    ke = min(kc + P, fin)
                    t = cp.tile([ke - kc, fa], BF16, tag=f"W{l}_{kc}")
                    nc.sync.dma_start(t[:], d_W[l][kc:ke, :])
                    chunks.append(t)
                Wt.append(chunks)
                a = cp.tile([P, cfg['fout']], F32, tag=f"b{l}")
                nc.sync.dma_start(a[:], d_b[l][:, :])
                bt.append(a)
            actT = [cp.tile([P, SHARD], BF16, tag=f"actT_{j}", name=f"actT_{j}")
                    for j in range(2)]
            # alpha_dst (shared across layers): [WIN, NW*4] bf16
            ado_all = cp.tile([WIN, NW * 4], BF16, tag="ado_all", name="ado_all")

            def dense_phase(l, actT_in):
                cfg = LAYERS[l]
                fout, heads, row = cfg['fout'], cfg['heads'], cfg['row']
                tab_s = tabs[l][0]
                nchunks = len(Wt[l])
                fa = fout + 2 * heads
                for w in range(NW):
                    ph = pp.tile([WIN, fa], F32, tag="ph", bufs=2)
                    for kc in range(nchunks):
                        nc.tensor.matmul(
                            ph[:, :], lhsT=actT_in[kc][:, w * WIN:(w + 1) * WIN],
                            rhs=Wt[l][kc][:], start=(kc == 0), stop=(kc == nchunks - 1))
                    row_t = rowp.tile([P, row], BF16, tag="row")
                    # rows: [h | alpha_src | zero pad]
                    nc.scalar.activation(row_t[:WIN, 0:fout + heads],
                                         ph[:, 0:fout + heads], ACTF.Copy)
                    if row > fout + heads:
                        nc.vector.memset(row_t[:, fout + heads:row], 0.0)
                    if timing_reps and os.environ.get("GAT_EDGE_KNOB", "") == "agoff":
                        nc.sync.dma_start(tab_s[w * WIN:(w + 1) * WIN, :],
                                          row_t[:WIN, :])
                    elif timing_reps:
                        # mock-AG, spread across the pipeline: one broadcast
                        # DMA writes this window's rows to all 8 shard slots
                        # of tab_f (equivalent receive volume to a real
                        # AllGather).
                        tab_f = tabs[l][1]
                        dst = tab_f.rearrange("(s n) r -> n s r", s=C)[
                            w * WIN:(w + 1) * WIN, :, :]
                        nc.sync.dma_start(
                            dst, row_t[:WIN, :].rearrange(
                                "w (o r) -> w o r", o=1).broadcast_to(
                                (WIN, C, row)))
                    else:
                        nc.sync.dma_start(tab_s[w * WIN:(w + 1) * WIN, :],
                                          row_t[:WIN, :])
                    # alpha_dst local store [WIN, heads] per window
                    nc.scalar.activation(ado_all[:, w * 4:w * 4 + heads],
                                         ph[:, fout + heads:fa], ACTF.Copy)

            def allgather(l):
                tab_s, tab_f = tabs[l]
                if timing_reps:
                    pass  # folded into dense_phase's per-window writes
                else:
                    nc.gpsimd.collective_compute(
                        "AllGather", ALU.bypass,
                        replica_groups=[list(range(C))],
                        ins=[tab_s[:, :]], outs=[tab_f[:, :]])

            def edge_phase(l, actT_next):
                cfg = LAYERS[l]
                fout, heads, row = cfg['fout'], cfg['heads'], cfg['row']
                dh = fout // heads
                ncols = fout + heads
                tab_f = tabs[l][1]
                icol = ccol = 0
                knob = os.environ.get("GAT_EDGE_KNOB", "") if timing_reps else ""
                for w in range(NW):
                    kw = kws[w]
                    tw = kw // P
                    gw = gp.tile([P, tw * row], BF16, tag="gw")
                    gw3 = gw[:].rearrange("p (t e) -> p t e", e=row)
                    if knob == "nogather" and w > 0:
                        nc.vector.memset(gw[:, 0:1], 0.0)
                    elif knob == "smallgather":
                        nc.gpsimd.dma_gather(
                            gw3[:, 0:1, :], tab_f[:, :],
                            idx_sb[:, icol:icol + P // 16],
                            P, P, row, single_packet=False)
                    else:
                        nc.gpsimd.dma_gather(
                            gw3, tab_f[:, :],
                            idx_sb[:, icol:icol + kw // 16],
                            kw, kw, row, single_packet=False)
                    # per-edge alpha_dst via one-hot^T matmul into PSUM
                    pads_ps = pp.tile([P, tw * 16], F32, tag="pads", bufs=2)
                    ado_w = ado_all[:, w * 4:w * 4 + heads]
                    for t in range(tw if knob != "nopads" else 1):
                        nc.tensor.matmul(
                            pads_ps[:, t * 16:t * 16 + heads],
                            lhsT=ohT_sb[:WIN, (ccol + t) * P:(ccol + t + 1) * P],
                            rhs=ado_w, start=True, stop=True)
                    if knob == "nopads":
                        nc.vector.memset(pads_ps[:, 16:], 0.0)
                    pads3 = pads_ps[:].rearrange("p (t e) -> p t e", e=16)
                    # scores: st = alpha_src + alpha_dst; lt = leaky(st)
                    st = sp.tile([P, tw * heads], F32, tag="st")
                    st3 = st[:].rearrange("p (t h) -> p t h", h=heads)
                    nc.vector.tensor_tensor(
                        st3, gw3[:, :, fout:fout + heads], pads3[:, :, 0:heads],
                        op=ALU.add)
                    lt = sp.tile([P, tw * heads], F32, tag="lt")
                    nc.vector.scalar_tensor_tensor(lt[:], st[:], NEG, st[:],
                                                   op0=ALU.mult, op1=ALU.max)
                    rhsW = rp.tile([P, tw * ncols], BF16, tag="rhsW")
                    rhs3 = rhsW[:].rearrange("p (t e) -> p t e", e=ncols)
                    nc.scalar.activation(
                        rhs3[:, :, fout:fout + heads],
                        lt[:].rearrange("p (t h) -> p t h", h=heads), ACTF.Exp)
                    # weighted messages ((d,h)-interleaved): rhs[:,:,0:fout] = h*p
                    if knob == "nowmult":
                        nc.scalar.activation(rhs3[:, :, 0:fout],
                                             gw3[:, :, 0:fout], ACTF.Copy)
                    else:
                        nc.vector.tensor_tensor(
                            rhs3[:, :, 0:fout].rearrange(
                                "p t (d h) -> p t d h", h=heads),
                            gw3[:, :, 0:fout].rearrange(
                                "p t (d h) -> p t d h", h=heads),
                            rhs3[:, :, fout:fout + heads].rearrange(
                                "p t (o h) -> p t o h", o=1).broadcast_to(
                                (P, tw, dh, heads)),
                            op=ALU.mult)
                    psw = pp.tile([WIN, ncols], F32, tag="psw", bufs=3)
                    ntw = tw if knob != "nopsw" else 1
                    for t in range(ntw):
                        nc.tensor.matmul(
                            psw[:], lhsT=oh_sb[:, (ccol + t) * WIN:(ccol + t + 1) * WIN],
                            rhs=rhs3[:, t, :], start=(t == 0), stop=(t == ntw - 1))
                    # window epilogue
                    den = sp.tile([WIN, heads], F32, tag="den")
                    nc.vector.tensor_scalar(den[:], psw[:, fout:fout + heads], 1e-16,
                                            None, op0=ALU.add)
                    rec = sp.tile([WIN, heads], F32, tag="rec")
                    nc.vector.reciprocal(rec[:], den[:])
                    if l < 2:
                        orow = op_.tile([P, fout], BF16, tag="orow")
                        # alpha-divide + bias + relu in (d,h) space
                        o3 = orow[:WIN, :].rearrange("p (d h) -> p d h", h=heads)
                        nc.vector.tensor_tensor(
                            o3, psw[:, 0:fout].rearrange("p (d h) -> p d h", h=heads),
                            rec[:].rearrange("p (o h) -> p o h", o=1).broadcast_to(
                                (WIN, dh, heads)),
                            op=ALU.mult)
                        nc.vector.tensor_tensor(orow[:WIN, :], orow[:WIN, :],
                                                bt[l][:WIN, :], op=ALU.add)
                        nc.vector.tensor_scalar(orow[:WIN, :], orow[:WIN, :], 0.0,
                                                None, op0=ALU.max)
                        for j in range(fout // P):
                            pt = pp.tile([P, WIN], BF16, tag="pt", bufs=1)
                            nc.tensor.transpose(pt[:], orow[:WIN, j * P:(j + 1) * P],
                                                identb[:WIN, :WIN])
                            nc.scalar.activation(
                                actT_next[j][:, w * WIN:(w + 1) * WIN], pt[:],
                                ACTF.Copy)
                    else:
                        orow = op_.tile([P, fout], F32, tag="orowf")
                        nc.vector.scalar_tensor_tensor(
                            orow[:WIN, :], psw[:, 0:fout], rec[:, 0:1],
                            bt[l][:WIN, :], op0=ALU.mult, op1=ALU.add)
                        nc.sync.dma_start(d_out[w * WIN:(w + 1) * WIN, :],
                                          orow[:WIN, :fout])
                    icol += kw // 16
                    ccol += tw

            def body():
                stages = int(os.environ.get("GAT_STAGES", "9"))  # 9 = full network
                dense_phase(0, [xT])
                if stages >= 2:
                    allgather(0)
                if stages >= 3:
                    edge_phase(0, actT)
                if stages >= 4:
                    dense_phase(1, actT)
                    allgather(1)
                if stages >= 5:
                    edge_phase(1, actT)
                if stages >= 6:
                    dense_phase(2, actT)
                    allgather(2)
                if stages >= 7:
                    edge_phase(2, None)
                if stages < 7:
                    z = op_.tile([WIN, OUT_CH], F32, tag="z", name="z")
                    nc.vector.memset(z[:], 0.0)
                    for w in range(NW):
                        nc.sync.dma_start(d_out[w * WIN:(w + 1) * WIN, :], z[:])

            if timing_reps:
                tk = cp.tile([1, 32], F32)
                nc.sync.dma_start(tk[:], d_tok[:, :])
                if timing_reps == 1:
                    body()
                else:
                    with tc.For_i(0, timing_reps, 1):
                        body()
                nc.sync.dma_start(d_toko[:, :], tk[:])
            else:
                body()

    nc.compile()
    return nc


def _augment_W(W, a_s, a_d, heads, fout):
    """[W | W@As_blockdiag | W@Ad_blockdiag] in fp32, cast bf16."""
    W = np.asarray(W, np.float64)
    a_s = np.asarray(a_s, np.float64).reshape(heads, fout // heads)
    a_d = np.asarray(a_d, np.float64).reshape(heads, fout // heads)
    dh = fout // heads
    Was = np.zeros((W.shape[0], heads))
    Wad = np.zeros((W.shape[0], heads))
    for h in range(heads):
        Was[:, h] = W[:, h * dh:(h + 1) * dh] @ a_s[h]
        Wad[:, h] = W[:, h * dh:(h + 1) * dh] @ a_d[h]
    return np.concatenate([W, Was, Wad], axis=1).astype(np.float32)


def _host_inputs(x, edge_index, W1, a1s, a1d, b1, W2, a2s, a2d, b2, W3, a3s, a3d, b3):
    kws, idx_all, oh_all, ohT_all = _host_prep(edge_index)
    x = np.asarray(x, dtype=np.float32)
    Wa = [_augment_W(W1, a1s, a1d, 4, 256), _augment_W(W2, a2s, a2d, 4, 256),
          _augment_W(W3, a3s, a3d, 1, 64)]
    bs = [np.asarray(b1, np.float32), np.asarray(b2, np.float32),
          np.asarray(b3, np.float32)]
    # (h*64+d) -> (d*4+h) interleave permutation for 4-head layers
    perm = np.empty(256, dtype=np.int64)
    for h in range(4):
        for d in range(64):
            perm[d * 4 + h] = h * 64 + d
    # layer-1/2 output columns interleaved; layer-2/3 input rows follow suit
    Wa[0][:, 0:256] = Wa[0][:, perm]
    Wa[1] = Wa[1][perm, :]
    Wa[1][:, 0:256] = Wa[1][:, perm]
    Wa[2] = Wa[2][perm, :]
    bs[0] = bs[0][perm]
    bs[1] = bs[1][perm]
    import ml_dtypes
    shared = {}
    for l in range(3):
        fout = LAYERS[l]['fout']
        shared[f"W{l+1}a"] = Wa[l].astype(ml_dtypes.bfloat16)
        shared[f"br{l+1}"] = np.tile(bs[l].reshape(1, fout), (P, 1))
    shared["identb"] = np.eye(P, dtype=np.float32).astype(ml_dtypes.bfloat16)
    in_maps = []
    for c in range(C):
        m = dict(shared)
        m["xT_own"] = np.ascontiguousarray(
            x[c * SHARD:(c + 1) * SHARD].T).astype(ml_dtypes.bfloat16)
        m["gat_idx"] = idx_all[c]
        m["oh_all"] = oh_all[c].astype(ml_dtypes.float8_e4m3)
        m["ohT_all"] = ohT_all[c].astype(ml_dtypes.float8_e4m3)
        in_maps.append(m)
    return kws, in_maps


_CACHE = {}


def kernel(**inputs) -> np.ndarray:
    kws, in_maps = _host_inputs(**inputs)
    if kws not in _CACHE:
        _CACHE[kws] = build(kws)
    nc = _CACHE[kws]
    last = None
    for _attempt in range(2):
        try:
            res = bass_utils.run_bass_kernel_spmd(
                nc, in_maps, core_ids=list(range(C)), trace=False)
            return np.concatenate(
                [res.results[c]["out"] for c in range(C)], axis=0)
        except Exception as e:  # rare transient device-mesh hiccups: retry once
            last = e
    raise last
